# revision 12
# baseline (speedup 1.0000x reference)
# Binary (sign) matmul: out[b,m,n] = sum_k sign(x[b,m,k]) * sign(y[b,n,k]) * x_clip * y_clip
# B=2, M=N=K=4096, fp32 in/out.
#
# Sharding: 8 cores = batch(2) x 2x2 grid over (M, N). Each core computes a
# [2048, 2048] output block from x[b, mh*2048:, :] and y[b, nh*2048:, :].
#
# Host marshalling: only the sign-carrying high byte of each fp32 input
# element is shipped to the device (a pure byte-slice view + transpose —
# no arithmetic); the device computes sign() from it (the int8 byte is
# negative iff the fp32 value is negative) via the Sign activation, then
# runs fp8 DoubleRow matmuls. This cuts input DMA 4x (16MB/core vs 64MB)
# so the PE — not the DMA stream — is the pacing engine.
#
# Per-core device pipeline:
#   DMA int8 k-major sign bytes -> ScalarE Sign (int8 -> fp8e4 +-1, into
#   the matmul operand buffers) -> TensorE DoubleRow fp8 matmuls (exact:
#   sums of +-1 accumulate in fp32 PSUM) -> quarter-K spills: partial sums
#   accumulate exactly in an fp16 SBUF tensor (every partial is an even
#   integer <= 4096, exactly representable) -> DVE add + ScalarE clip
#   scale -> DMA out.
#
# Matmul schedule: K is split into 4 quarters of 4 DoubleRow steps. Each
# (m-tile, quarter) is one PSUM accumulation group of 4 banks; quarters
# are processed outer-to-inner in k-arrival order so the in-order PE queue
# never waits on a chunk that arrives later than runnable work. With only
# ~1/4 of K needed before the first matmuls close, the PE stays busy from
# ~30us onward instead of idling ~100us against the input stream.
import numpy as np

B = 2
M = N = K = 4096
P = 128
MSH, NSH = 2048, 2048      # per-core shard of M, N
KO = K // P                # 32 k-tiles of 128
MT = MSH // P              # 16 m row-tiles
FD = 512                   # matmul free dim
NCH = NSH // FD            # 4 n chunks
NCORES = 8
# K split into PSUM accumulation phases per m-tile: a 3-step and a 5-step
# phase (spilled exactly to fp16 ACC in k-arrival order so the PE never
# waits on the input stream; the short first phase keeps a safety margin
# over the Sign stream, which delivers one k-chunk per ~1.9us), then one
# 8-step half drained straight to the output (half as many drains so the
# out-DMA chain hides under the matmuls).
PHASES = ((0, 3), (3, 5), (8, 8))  # (kd start, kd count)


def _build_program():
    import concourse.bacc as bacc
    import concourse.mybir as mybir
    import concourse.tile as tile
    from concourse.bass import ts

    f32 = mybir.dt.float32
    f16 = mybir.dt.float16
    bf16 = mybir.dt.bfloat16
    i8 = mybir.dt.int8
    op_dt = mybir.dt.float8e4
    Sign = mybir.ActivationFunctionType.Sign
    Copy = mybir.ActivationFunctionType.Copy

    # Bacc (not bass.Bass): its compile() legalizes multi-sem waits into
    # event-semaphore carriers — TRN2 instructions support only 1 HW wait.
    nc = bacc.Bacc(
        "TRN2",
        target_bir_lowering=False,
        debug=False,
        num_devices=NCORES,
    )
    xsT = nc.dram_tensor("xsT", [K, MSH], i8, kind="ExternalInput").ap()
    ysT = nc.dram_tensor("ysT", [K, NSH], i8, kind="ExternalInput").ap()
    clips = nc.dram_tensor("clips", [P, 2], f32, kind="ExternalInput").ap()
    out = nc.dram_tensor("out", [MSH, NSH], f32, kind="ExternalOutput").ap()

    with tile.TileContext(nc) as tc:
        with (
            tc.tile_pool(name="constp", bufs=1) as constp,
            tc.tile_pool(name="sytp", bufs=1) as sytp,
            tc.tile_pool(name="sxtp", bufs=1) as sxtp,
            tc.tile_pool(name="accp", bufs=1) as accp,
            tc.tile_pool(name="stagep", bufs=4) as stagep,
            tc.tile_pool(name="outp", bufs=3) as outp,
            tc.tile_pool(name="psump", bufs=7, space="PSUM") as psump,
            tc.tile_pool(name="dpsump", bufs=1, space="PSUM") as dpsump,
        ):
            # clip product, replicated per-partition: [P, 1]
            clip_sb = constp.tile([P, 2], f32)
            nc.sync.dma_start(clip_sb[:], clips)
            clip_prod = constp.tile([P, 1], f32)
            nc.vector.tensor_tensor(
                clip_prod[:], clip_sb[:, 0:1], clip_sb[:, 1:2],
                mybir.AluOpType.mult,
            )

            # SxT[ki, ko, m] = sign(x[m, ko*P + ki]); SyT likewise for y.
            SxT = sxtp.tile([P, KO, MSH], op_dt)
            SyT = sytp.tile([P, KO, NSH], op_dt)

            # HAM warmth: DoubleRow matmuls don't register as PE activity,
            # so the clock gate re-throttles to 1.2 GHz mid-stream. A tiny
            # normal-mode matmul on scratch data every couple of k-steps
            # keeps the activity monitor busy; it clobbers 32 stationary
            # columns, so it only ever sits at k-step boundaries (weights
            # reload anyway).
            dwarm = constp.tile([P, 32], bf16)
            nc.vector.memset(dwarm[:], 0)

            def warm_mm(kd=None):
                # The PE hoists dependency-free instructions through its
                # 64-deep reorder window, which would bunch all warm matmuls
                # at the head of the kernel and break the cadence. Anchoring
                # the moving operand to a just-signed SxT slice pins each
                # warm matmul to its intended neighborhood. (Garbage result
                # into scratch PSUM; mixed bf16/fp8 operands are legal.)
                rhs = dwarm[:] if kd is None else SxT[:, 2 * kd, 0:32]
                dps = dpsump.tile([32, 32], f32, name="dps", tag="dps")
                nc.tensor.matmul(
                    dps[:], lhsT=dwarm[:, :32], rhs=rhs, start=True,
                    stop=True,
                )

            def prep(src_dram, ko, dst_sx, splits=None):
                st = stagep.tile([P, MSH], i8, name="st", tag="stage")
                nc.sync.dma_start(st[:], src_dram[ts(ko, P), :])
                acts = []
                if splits is None:
                    acts.append(lambda: nc.scalar.activation(dst_sx, st[:], Sign))
                else:
                    for lo, hi in splits:
                        acts.append(
                            lambda lo=lo, hi=hi: nc.scalar.activation(
                                dst_sx[:, lo:hi], st[:, lo:hi], Sign
                            )
                        )
                return acts

            # Stream k-chunks: x and y interleaved so the first matmuls can
            # start after one chunk of each; the Tile scheduler overlaps the
            # rest of the prep with the matmul wavefront. The first two
            # chunks of each operand are signed in two pieces, low columns
            # first, so the first matmuls' operands are ready ~4us earlier.
            head_acts = []
            for ko in range(2):
                head_acts.append(prep(xsT, ko, SxT[:, ko, :], (((0, 512), (512, MSH)))))
                head_acts.append(prep(ysT, ko, SyT[:, ko, :], (((0, 512), (512, MSH)))))
            for part in range(2):
                for acts in head_acts:
                    acts[part]()
            for ko in range(2, KO):
                prep(xsT, ko, SxT[:, ko, :])[0]()
                prep(ysT, ko, SyT[:, ko, :])[0]()

            # Dependency-free warm burst: fills the PE while the first
            # chunks are DMA'd + signed, ramping the p-state and the HAM
            # activity monitor before real work hits the array.
            for _ in range(28):
                warm_mm()

            # Exact fp16 accumulator for quarter partial sums: every spilled
            # value is an even integer of magnitude <= 4096.
            ACC = accp.tile([P, MT, NCH, FD], f16, name="ACC")

            kstep = 0
            for q, (kd0, nkd) in enumerate(PHASES):
                last = q == len(PHASES) - 1
                for i in range(MT):
                    pss = [
                        psump.tile([P, FD], f32, name=f"ps{n}", tag="ps")
                        for n in range(NCH)
                    ]
                    for kdq in range(nkd):
                        kd = kd0 + kdq
                        # One warm matmul every 3rd k-step (~3.1us) keeps the
                        # HAM clock gate inside its ~3.4us full-speed window
                        # at minimum PE cost.
                        if kstep % 3 == 2:
                            warm_mm(kd)
                        kstep += 1
                        # nch-inner: 4 consecutive matmuls share one
                        # stationary tile; _dedupe_ldweights drops the
                        # redundant loads.
                        for nch in range(NCH):
                            nc.tensor.matmul(
                                pss[nch][:],
                                lhsT=SxT[:, 2 * kd : 2 * kd + 2, ts(i, P)],
                                rhs=SyT[:, 2 * kd : 2 * kd + 2, ts(nch, FD)],
                                start=(kdq == 0),
                                stop=(kdq == nkd - 1),
                                perf_mode=mybir.MatmulPerfMode.DoubleRow,
                            )
                    for nch in range(NCH):
                        if q == 0:
                            nc.vector.tensor_copy(
                                out=ACC[:, i, nch, :], in_=pss[nch][:]
                            )
                        elif not last:
                            nc.vector.tensor_tensor(
                                ACC[:, i, nch, :], pss[nch][:],
                                ACC[:, i, nch, :], mybir.AluOpType.add,
                            )
                        else:
                            ot = outp.tile([P, FD], f32, name="ot")
                            nc.vector.tensor_tensor(
                                ot[:], pss[nch][:], ACC[:, i, nch, :],
                                mybir.AluOpType.add,
                            )
                            # clip scale on the Scalar engine (its queue is
                            # idle once the signs are done); DVE only does
                            # the adds.
                            nc.scalar.activation(
                                ot[:], ot[:], Copy, scale=clip_prod[:]
                            )
                            nc.sync.dma_start(
                                out[ts(i, P), ts(nch, FD)], ot[:]
                            )

    nc.compile()
    _dedupe_ldweights(nc)
    return nc


def _dedupe_ldweights(nc):
    """Drop redundant standalone InstLdweights left by bacc's matmul split.

    With the nch-inner loop order, 4 consecutive matmuls share one
    stationary tile; bacc still emits one InstLdweights per matmul. An
    InstLdweights identical to the previous one (same AP, same mode) with
    no semaphore waits/updates is a no-op — remove it."""
    removed = 0
    for blk in nc.m.functions[0].blocks:
        prev_key = None
        warm_seen = False
        keep = []
        for inst in blk.instructions:
            nm = type(inst).__name__
            if nm == "InstLdweights":
                pap = inst.ins[0]
                key = (
                    pap.memref,
                    pap.offset,
                    str(pap.ap),
                    str(pap.dtype),
                    str(inst.perf_mode),
                    str(inst.is_transpose),
                )
                is_warm = "bfloat16" in str(pap.dtype)
                if (
                    key == prev_key
                    and not inst.has_wait()
                    and not inst.has_update()
                ):
                    # Identical to the previous load — or a warm-matmul
                    # weight reload: the warm matmul's output is scratch and
                    # its moving data is zeros, so it can run on whatever
                    # weights are resident; skipping the reload avoids
                    # clobbering the background weight buffer and exposing a
                    # serial LDWEIGHTS on the next real matmul.
                    removed += 1
                    continue
                if is_warm:
                    warm_seen = True
                prev_key = key
            keep.append(inst)
        if removed:
            blk.instructions = keep
    return removed


_PROGRAM_CACHE = None
_LDW_PATCHED = False


def _patch_ldw_opt():
    """Re-enable walrus's LDWEIGHTS elision (consecutive matmuls sharing a
    stationary tile skip the reload). bass_utils hardcodes it off."""
    global _LDW_PATCHED
    if _LDW_PATCHED:
        return
    import concourse.bass_utils as _bu

    _orig = _bu.run_command

    def _run(argv, **kwargs):
        if isinstance(argv, list):
            argv = [
                "--enable-ldw-opt=true" if a == "--enable-ldw-opt=false" else a
                for a in argv
            ]
        return _orig(argv, **kwargs)

    _bu.run_command = _run
    _LDW_PATCHED = True


def _get_program():
    global _PROGRAM_CACHE
    if _PROGRAM_CACHE is None:
        _PROGRAM_CACHE = _build_program()
    return _PROGRAM_CACHE


def _sign_bytes(a):
    """[R, K] fp32 -> [K, R] int8 view of each element's top byte (sign +
    high exponent bits). Negative byte <=> negative float. Pure memory
    marshalling — byte selection + transpose, no arithmetic."""
    b = a.view(np.uint8).reshape(a.shape[0], a.shape[1], 4)[:, :, 3]
    return np.ascontiguousarray(b.view(np.int8).T)


def _shard_inputs(x, y, x_clip, y_clip):
    x = np.ascontiguousarray(np.asarray(x, dtype=np.float32))
    y = np.ascontiguousarray(np.asarray(y, dtype=np.float32))
    clips = np.empty((P, 2), dtype=np.float32)
    clips[:, 0] = np.float32(x_clip)
    clips[:, 1] = np.float32(y_clip)
    xb = {}
    yb = {}
    for bb in range(B):
        for h in range(2):
            xb[(bb, h)] = _sign_bytes(x[bb, h * MSH : (h + 1) * MSH, :])
            yb[(bb, h)] = _sign_bytes(y[bb, h * NSH : (h + 1) * NSH, :])
    in_maps = []
    for c in range(NCORES):
        b, mh, nh = c // 4, (c % 4) // 2, c % 2
        in_maps.append(
            {"xsT": xb[(b, mh)], "ysT": yb[(b, nh)], "clips": clips}
        )
    return in_maps


def run_sharded(x, y, x_clip, y_clip, trace=False, **kwargs):
    """Run the SPMD kernel; returns (out, BassKernelResults)."""
    from concourse.bass_utils import run_bass_kernel_spmd

    nc = _get_program()
    in_maps = _shard_inputs(x, y, x_clip, y_clip)
    res = run_bass_kernel_spmd(
        nc, in_maps, core_ids=list(range(NCORES)), trace=trace, **kwargs
    )
    out = np.empty((B, M, N), dtype=np.float32)
    for c in range(NCORES):
        b, mh, nh = c // 4, (c % 4) // 2, c % 2
        out[b, mh * MSH : (mh + 1) * MSH, nh * NSH : (nh + 1) * NSH] = res.results[
            c
        ]["out"]
    return out, res


def kernel(x, y, x_clip, y_clip):
    out, _ = run_sharded(x, y, x_clip, y_clip, trace=False)
    return out


# revision 13
# speedup vs baseline: 1.0505x; 1.0505x over previous
# Binary (sign) matmul: out[b,m,n] = sum_k sign(x[b,m,k]) * sign(y[b,n,k]) * x_clip * y_clip
# B=2, M=N=K=4096, fp32 in/out.
#
# Sharding: 8 cores = batch(2) x 2x2 grid over (M, N). Each core computes a
# [2048, 2048] output block from x[b, mh*2048:, :] and y[b, nh*2048:, :].
#
# Host marshalling: only the sign-carrying high byte of each fp32 input
# element is shipped to the device (a pure byte-slice view + transpose —
# no arithmetic); the device computes sign() from it (the int8 byte is
# negative iff the fp32 value is negative) via the Sign activation, then
# runs fp8 DoubleRow matmuls. This cuts input DMA 4x (16MB/core vs 64MB)
# so the PE — not the DMA stream — is the pacing engine.
#
# Per-core device pipeline:
#   DMA int8 k-major sign bytes -> ScalarE Sign (int8 -> fp8e4 +-1, into
#   the matmul operand buffers) -> TensorE DoubleRow fp8 matmuls (exact:
#   sums of +-1 accumulate in fp32 PSUM) -> quarter-K spills: partial sums
#   accumulate exactly in an fp16 SBUF tensor (every partial is an even
#   integer <= 4096, exactly representable) -> DVE add + ScalarE clip
#   scale -> DMA out.
#
# Matmul schedule: K is split into 4 quarters of 4 DoubleRow steps. Each
# (m-tile, quarter) is one PSUM accumulation group of 4 banks; quarters
# are processed outer-to-inner in k-arrival order so the in-order PE queue
# never waits on a chunk that arrives later than runnable work. With only
# ~1/4 of K needed before the first matmuls close, the PE stays busy from
# ~30us onward instead of idling ~100us against the input stream.
import numpy as np

B = 2
M = N = K = 4096
P = 128
MSH, NSH = 2048, 2048      # per-core shard of M, N
KO = K // P                # 32 k-tiles of 128
MT = MSH // P              # 16 m row-tiles
FD = 512                   # matmul free dim
NCH = NSH // FD            # 4 n chunks
NCORES = 8
# K split into PSUM accumulation phases per m-tile: two 4-step quarters
# (spilled exactly to fp16 ACC in k-arrival order so the PE never waits on
# the input stream), then one 8-step half (drained straight to the output,
# half as many drains so the out-DMA chain hides under the matmuls).
PHASES = ((0, 4), (4, 4), (8, 8))  # (kd start, kd count)


def _build_program():
    import concourse.bacc as bacc
    import concourse.mybir as mybir
    import concourse.tile as tile
    from concourse.bass import ts

    f32 = mybir.dt.float32
    f16 = mybir.dt.float16
    bf16 = mybir.dt.bfloat16
    i8 = mybir.dt.int8
    op_dt = mybir.dt.float8e4
    Sign = mybir.ActivationFunctionType.Sign
    Copy = mybir.ActivationFunctionType.Copy

    # Bacc (not bass.Bass): its compile() legalizes multi-sem waits into
    # event-semaphore carriers — TRN2 instructions support only 1 HW wait.
    nc = bacc.Bacc(
        "TRN2",
        target_bir_lowering=False,
        debug=False,
        num_devices=NCORES,
    )
    xsT = nc.dram_tensor("xsT", [K, MSH], i8, kind="ExternalInput").ap()
    ysT = nc.dram_tensor("ysT", [K, NSH], i8, kind="ExternalInput").ap()
    clips = nc.dram_tensor("clips", [P, 2], f32, kind="ExternalInput").ap()
    out = nc.dram_tensor("out", [MSH, NSH], f32, kind="ExternalOutput").ap()

    with tile.TileContext(nc) as tc:
        with (
            tc.tile_pool(name="constp", bufs=1) as constp,
            tc.tile_pool(name="sytp", bufs=1) as sytp,
            tc.tile_pool(name="sxtp", bufs=1) as sxtp,
            tc.tile_pool(name="accp", bufs=1) as accp,
            tc.tile_pool(name="stagep", bufs=4) as stagep,
            tc.tile_pool(name="outp", bufs=3) as outp,
            tc.tile_pool(name="psump", bufs=7, space="PSUM") as psump,
            tc.tile_pool(name="dpsump", bufs=1, space="PSUM") as dpsump,
        ):
            # clip product, replicated per-partition: [P, 1]
            clip_sb = constp.tile([P, 2], f32)
            nc.sync.dma_start(clip_sb[:], clips)
            clip_prod = constp.tile([P, 1], f32)
            nc.vector.tensor_tensor(
                clip_prod[:], clip_sb[:, 0:1], clip_sb[:, 1:2],
                mybir.AluOpType.mult,
            )

            # SxT[ki, ko, m] = sign(x[m, ko*P + ki]); SyT likewise for y.
            SxT = sxtp.tile([P, KO, MSH], op_dt)
            SyT = sytp.tile([P, KO, NSH], op_dt)

            # HAM warmth: DoubleRow matmuls don't register as PE activity,
            # so the clock gate re-throttles to 1.2 GHz mid-stream. A tiny
            # normal-mode matmul on scratch data every couple of k-steps
            # keeps the activity monitor busy; it clobbers 32 stationary
            # columns, so it only ever sits at k-step boundaries (weights
            # reload anyway).
            dwarm = constp.tile([P, 32], bf16)
            nc.vector.memset(dwarm[:], 0)

            def warm_mm():
                dps = dpsump.tile([32, 32], f32, name="dps", tag="dps")
                nc.tensor.matmul(
                    dps[:], lhsT=dwarm[:, :32], rhs=dwarm[:], start=True,
                    stop=True,
                )

            def prep(src_dram, ko, dst):
                st = stagep.tile([P, MSH], i8, name="st", tag="stage")
                nc.sync.dma_start(st[:], src_dram[ts(ko, P), :])
                nc.scalar.activation(dst, st[:], Sign)

            # Stream k-chunks: x and y interleaved so the first matmuls can
            # start after one chunk of each; the Tile scheduler overlaps the
            # rest of the prep with the matmul wavefront.
            for ko in range(KO):
                prep(xsT, ko, SxT[:, ko, :])
                prep(ysT, ko, SyT[:, ko, :])

            # Ramp the PE p-state before real work hits it.
            for _ in range(6):
                warm_mm()

            # Exact fp16 accumulator for quarter partial sums: every spilled
            # value is an even integer of magnitude <= 4096.
            ACC = accp.tile([P, MT, NCH, FD], f16, name="ACC")

            kstep = 0
            for q, (kd0, nkd) in enumerate(PHASES):
                last = q == len(PHASES) - 1
                for i in range(MT):
                    pss = [
                        psump.tile([P, FD], f32, name=f"ps{n}", tag="ps")
                        for n in range(NCH)
                    ]
                    for kdq in range(nkd):
                        kd = kd0 + kdq
                        # One warm matmul every 3rd k-step (~3.1us) keeps the
                        # HAM clock gate inside its ~3.4us full-speed window
                        # at minimum PE cost.
                        if kstep % 3 == 2:
                            warm_mm()
                        kstep += 1
                        # nch-inner: 4 consecutive matmuls share one
                        # stationary tile; _dedupe_ldweights drops the
                        # redundant loads.
                        for nch in range(NCH):
                            nc.tensor.matmul(
                                pss[nch][:],
                                lhsT=SxT[:, 2 * kd : 2 * kd + 2, ts(i, P)],
                                rhs=SyT[:, 2 * kd : 2 * kd + 2, ts(nch, FD)],
                                start=(kdq == 0),
                                stop=(kdq == nkd - 1),
                                perf_mode=mybir.MatmulPerfMode.DoubleRow,
                            )
                    for nch in range(NCH):
                        if q == 0:
                            nc.vector.tensor_copy(
                                out=ACC[:, i, nch, :], in_=pss[nch][:]
                            )
                        elif not last:
                            nc.vector.tensor_tensor(
                                ACC[:, i, nch, :], pss[nch][:],
                                ACC[:, i, nch, :], mybir.AluOpType.add,
                            )
                        else:
                            ot = outp.tile([P, FD], f32, name="ot")
                            nc.vector.tensor_tensor(
                                ot[:], pss[nch][:], ACC[:, i, nch, :],
                                mybir.AluOpType.add,
                            )
                            # clip scale on the Scalar engine (its queue is
                            # idle once the signs are done); DVE only does
                            # the adds.
                            nc.scalar.activation(
                                ot[:], ot[:], Copy, scale=clip_prod[:]
                            )
                            nc.sync.dma_start(
                                out[ts(i, P), ts(nch, FD)], ot[:]
                            )

    nc.compile()
    _dedupe_ldweights(nc)
    return nc


def _dedupe_ldweights(nc):
    """Drop redundant standalone InstLdweights left by bacc's matmul split.

    With the nch-inner loop order, 4 consecutive matmuls share one
    stationary tile; bacc still emits one InstLdweights per matmul. An
    InstLdweights identical to the previous one (same AP, same mode) with
    no semaphore waits/updates is a no-op — remove it."""
    removed = 0
    for blk in nc.m.functions[0].blocks:
        prev_key = None
        warm_seen = False
        keep = []
        for inst in blk.instructions:
            nm = type(inst).__name__
            if nm == "InstLdweights":
                pap = inst.ins[0]
                key = (
                    pap.memref,
                    pap.offset,
                    str(pap.ap),
                    str(pap.dtype),
                    str(inst.perf_mode),
                    str(inst.is_transpose),
                )
                is_warm = "bfloat16" in str(pap.dtype)
                if (
                    key == prev_key
                    and not inst.has_wait()
                    and not inst.has_update()
                ):
                    # Identical to the previous load — or a warm-matmul
                    # weight reload: the warm matmul's output is scratch and
                    # its moving data is zeros, so it can run on whatever
                    # weights are resident; skipping the reload avoids
                    # clobbering the background weight buffer and exposing a
                    # serial LDWEIGHTS on the next real matmul.
                    removed += 1
                    continue
                if is_warm:
                    warm_seen = True
                prev_key = key
            keep.append(inst)
        if removed:
            blk.instructions = keep
    return removed


_PROGRAM_CACHE = None
_LDW_PATCHED = False


def _patch_ldw_opt():
    """Re-enable walrus's LDWEIGHTS elision (consecutive matmuls sharing a
    stationary tile skip the reload). bass_utils hardcodes it off."""
    global _LDW_PATCHED
    if _LDW_PATCHED:
        return
    import concourse.bass_utils as _bu

    _orig = _bu.run_command

    def _run(argv, **kwargs):
        if isinstance(argv, list):
            argv = [
                "--enable-ldw-opt=true" if a == "--enable-ldw-opt=false" else a
                for a in argv
            ]
        return _orig(argv, **kwargs)

    _bu.run_command = _run
    _LDW_PATCHED = True


def _get_program():
    global _PROGRAM_CACHE
    if _PROGRAM_CACHE is None:
        _PROGRAM_CACHE = _build_program()
    return _PROGRAM_CACHE


def _sign_bytes(a):
    """[R, K] fp32 -> [K, R] int8 view of each element's top byte (sign +
    high exponent bits). Negative byte <=> negative float. Pure memory
    marshalling — byte selection + transpose, no arithmetic."""
    b = a.view(np.uint8).reshape(a.shape[0], a.shape[1], 4)[:, :, 3]
    return np.ascontiguousarray(b.view(np.int8).T)


def _shard_inputs(x, y, x_clip, y_clip):
    x = np.ascontiguousarray(np.asarray(x, dtype=np.float32))
    y = np.ascontiguousarray(np.asarray(y, dtype=np.float32))
    clips = np.empty((P, 2), dtype=np.float32)
    clips[:, 0] = np.float32(x_clip)
    clips[:, 1] = np.float32(y_clip)
    xb = {}
    yb = {}
    for bb in range(B):
        for h in range(2):
            xb[(bb, h)] = _sign_bytes(x[bb, h * MSH : (h + 1) * MSH, :])
            yb[(bb, h)] = _sign_bytes(y[bb, h * NSH : (h + 1) * NSH, :])
    in_maps = []
    for c in range(NCORES):
        b, mh, nh = c // 4, (c % 4) // 2, c % 2
        in_maps.append(
            {"xsT": xb[(b, mh)], "ysT": yb[(b, nh)], "clips": clips}
        )
    return in_maps


def run_sharded(x, y, x_clip, y_clip, trace=False, **kwargs):
    """Run the SPMD kernel; returns (out, BassKernelResults)."""
    from concourse.bass_utils import run_bass_kernel_spmd

    nc = _get_program()
    in_maps = _shard_inputs(x, y, x_clip, y_clip)
    res = run_bass_kernel_spmd(
        nc, in_maps, core_ids=list(range(NCORES)), trace=trace, **kwargs
    )
    out = np.empty((B, M, N), dtype=np.float32)
    for c in range(NCORES):
        b, mh, nh = c // 4, (c % 4) // 2, c % 2
        out[b, mh * MSH : (mh + 1) * MSH, nh * NSH : (nh + 1) * NSH] = res.results[
            c
        ]["out"]
    return out, res


def kernel(x, y, x_clip, y_clip):
    out, _ = run_sharded(x, y, x_clip, y_clip, trace=False)
    return out


# revision 16
# speedup vs baseline: 1.0709x; 1.0194x over previous
# Binary (sign) matmul: out[b,m,n] = sum_k sign(x[b,m,k]) * sign(y[b,n,k]) * x_clip * y_clip
# B=2, M=N=K=4096, fp32 in/out.
#
# Sharding: 8 cores = batch(2) x 2x2 grid over (M, N). Each core computes a
# [2048, 2048] output block from x[b, mh*2048:, :] and y[b, nh*2048:, :].
#
# Host marshalling: only the sign-carrying high byte of each fp32 input
# element is shipped to the device (a pure byte-slice view + transpose —
# no arithmetic); the device computes sign() from it (the int8 byte is
# negative iff the fp32 value is negative) via the Sign activation, then
# runs fp8 DoubleRow matmuls. This cuts input DMA 4x (16MB/core vs 64MB)
# so the PE — not the DMA stream — is the pacing engine.
#
# Per-core device pipeline:
#   DMA int8 k-major sign bytes -> ScalarE Sign (int8 -> fp8e4 +-1, into
#   the matmul operand buffers) -> TensorE DoubleRow fp8 matmuls (exact:
#   sums of +-1 accumulate in fp32 PSUM) -> quarter-K spills: partial sums
#   accumulate exactly in an fp16 SBUF tensor (every partial is an even
#   integer <= 4096, exactly representable) -> DVE add + ScalarE clip
#   scale -> DMA out.
#
# Matmul schedule: K is split into 4 quarters of 4 DoubleRow steps. Each
# (m-tile, quarter) is one PSUM accumulation group of 4 banks; quarters
# are processed outer-to-inner in k-arrival order so the in-order PE queue
# never waits on a chunk that arrives later than runnable work. With only
# ~1/4 of K needed before the first matmuls close, the PE stays busy from
# ~30us onward instead of idling ~100us against the input stream.
import numpy as np

B = 2
M = N = K = 4096
P = 128
MSH, NSH = 2048, 2048      # per-core shard of M, N
KO = K // P                # 32 k-tiles of 128
MT = MSH // P              # 16 m row-tiles
FD = 512                   # matmul free dim
NCH = NSH // FD            # 4 n chunks
NCORES = 8
# K split into PSUM accumulation phases per m-tile: 4-step quarters
# (spilled exactly to fp16 ACC in k-arrival order so the PE never waits on
# the input stream), then one 8-step half (drained straight to the output,
# half as many drains so the out-DMA chain hides under the matmuls).
#
# The first FINE m-tiles additionally split their first quarter into two
# 2-step sub-phases: the PE's 64-deep reorder window can then reach enough
# runnable low-kd matmuls to cover the Sign stream's ramp (kd j is fully
# signed only at ~18+7.6j us), eliminating ~10us of early pinch stalls.
FINE = 6
SEGMENTS = (
    (0, FINE, 0, 2, "cast"),    # A1: fine tiles, kd 0-1
    (FINE, MT, 0, 4, "cast"),   # A:  remaining tiles, kd 0-3
    (0, FINE, 2, 2, "add"),     # A2: fine tiles, kd 2-3
    (0, MT, 4, 4, "add"),       # B:  all tiles, kd 4-7
    (0, MT, 8, 8, "final"),     # C:  all tiles, kd 8-15, drain to output
)


def _build_program():
    import concourse.bacc as bacc
    import concourse.mybir as mybir
    import concourse.tile as tile
    from concourse.bass import ts

    f32 = mybir.dt.float32
    f16 = mybir.dt.float16
    bf16 = mybir.dt.bfloat16
    i8 = mybir.dt.int8
    op_dt = mybir.dt.float8e4
    Sign = mybir.ActivationFunctionType.Sign
    Copy = mybir.ActivationFunctionType.Copy

    # Bacc (not bass.Bass): its compile() legalizes multi-sem waits into
    # event-semaphore carriers — TRN2 instructions support only 1 HW wait.
    nc = bacc.Bacc(
        "TRN2",
        target_bir_lowering=False,
        debug=False,
        num_devices=NCORES,
    )
    xsT = nc.dram_tensor("xsT", [K, MSH], i8, kind="ExternalInput").ap()
    ysT = nc.dram_tensor("ysT", [K, NSH], i8, kind="ExternalInput").ap()
    clips = nc.dram_tensor("clips", [P, 2], f32, kind="ExternalInput").ap()
    out = nc.dram_tensor("out", [MSH, NSH], f32, kind="ExternalOutput").ap()

    with tile.TileContext(nc) as tc:
        with (
            tc.tile_pool(name="constp", bufs=1) as constp,
            tc.tile_pool(name="sytp", bufs=1) as sytp,
            tc.tile_pool(name="sxtp", bufs=1) as sxtp,
            tc.tile_pool(name="accp", bufs=1) as accp,
            tc.tile_pool(name="stagep", bufs=4) as stagep,
            tc.tile_pool(name="outp", bufs=3) as outp,
            tc.tile_pool(name="psump", bufs=7, space="PSUM") as psump,
            tc.tile_pool(name="dpsump", bufs=1, space="PSUM") as dpsump,
        ):
            # clip product, replicated per-partition: [P, 1]
            clip_sb = constp.tile([P, 2], f32)
            nc.sync.dma_start(clip_sb[:], clips)
            clip_prod = constp.tile([P, 1], f32)
            nc.vector.tensor_tensor(
                clip_prod[:], clip_sb[:, 0:1], clip_sb[:, 1:2],
                mybir.AluOpType.mult,
            )

            # SxT[ki, ko, m] = sign(x[m, ko*P + ki]); SyT likewise for y.
            SxT = sxtp.tile([P, KO, MSH], op_dt)
            SyT = sytp.tile([P, KO, NSH], op_dt)

            # HAM warmth: DoubleRow matmuls don't register as PE activity,
            # so the clock gate re-throttles to 1.2 GHz mid-stream. A tiny
            # normal-mode matmul on scratch data every couple of k-steps
            # keeps the activity monitor busy; it clobbers 32 stationary
            # columns, so it only ever sits at k-step boundaries (weights
            # reload anyway).
            dwarm = constp.tile([P, 32], bf16)
            nc.vector.memset(dwarm[:], 0)

            def warm_mm():
                dps = dpsump.tile([32, 32], f32, name="dps", tag="dps")
                nc.tensor.matmul(
                    dps[:], lhsT=dwarm[:, :32], rhs=dwarm[:], start=True,
                    stop=True,
                )

            def prep(src_dram, ko, dst):
                st = stagep.tile([P, MSH], i8, name="st", tag="stage")
                nc.sync.dma_start(st[:], src_dram[ts(ko, P), :])
                nc.scalar.activation(dst, st[:], Sign)

            # Stream k-chunks: x and y interleaved so the first matmuls can
            # start after one chunk of each; the Tile scheduler overlaps the
            # rest of the prep with the matmul wavefront.
            for ko in range(KO):
                prep(xsT, ko, SxT[:, ko, :])
                prep(ysT, ko, SyT[:, ko, :])

            # Ramp the PE p-state before real work hits it.
            for _ in range(6):
                warm_mm()

            # Exact fp16 accumulator for quarter partial sums: every spilled
            # value is an even integer of magnitude <= 4096.
            ACC = accp.tile([P, MT, NCH, FD], f16, name="ACC")

            kstep = 0
            for i0, i1, kd0, nkd, kind in SEGMENTS:
                for i in range(i0, i1):
                    pss = [
                        psump.tile([P, FD], f32, name=f"ps{n}", tag="ps")
                        for n in range(NCH)
                    ]
                    for kdq in range(nkd):
                        kd = kd0 + kdq
                        # One warm matmul every 3rd k-step (~3.1us) keeps the
                        # HAM clock gate inside its ~3.4us full-speed window
                        # at minimum PE cost.
                        if kstep % 3 == 2:
                            warm_mm()
                        kstep += 1
                        # nch-inner: 4 consecutive matmuls share one
                        # stationary tile; _dedupe_ldweights drops the
                        # redundant loads.
                        for nch in range(NCH):
                            nc.tensor.matmul(
                                pss[nch][:],
                                lhsT=SxT[:, 2 * kd : 2 * kd + 2, ts(i, P)],
                                rhs=SyT[:, 2 * kd : 2 * kd + 2, ts(nch, FD)],
                                start=(kdq == 0),
                                stop=(kdq == nkd - 1),
                                perf_mode=mybir.MatmulPerfMode.DoubleRow,
                            )
                    for nch in range(NCH):
                        if kind == "cast":
                            nc.vector.tensor_copy(
                                out=ACC[:, i, nch, :], in_=pss[nch][:]
                            )
                        elif kind == "add":
                            nc.vector.tensor_tensor(
                                ACC[:, i, nch, :], pss[nch][:],
                                ACC[:, i, nch, :], mybir.AluOpType.add,
                            )
                        else:
                            ot = outp.tile([P, FD], f32, name="ot")
                            nc.vector.tensor_tensor(
                                ot[:], pss[nch][:], ACC[:, i, nch, :],
                                mybir.AluOpType.add,
                            )
                            # clip scale on the Scalar engine (its queue is
                            # idle once the signs are done); DVE only does
                            # the adds.
                            nc.scalar.activation(
                                ot[:], ot[:], Copy, scale=clip_prod[:]
                            )
                            nc.sync.dma_start(
                                out[ts(i, P), ts(nch, FD)], ot[:]
                            )

    nc.compile()
    _dedupe_ldweights(nc)
    return nc


def _dedupe_ldweights(nc):
    """Drop redundant standalone InstLdweights left by bacc's matmul split.

    With the nch-inner loop order, 4 consecutive matmuls share one
    stationary tile; bacc still emits one InstLdweights per matmul. An
    InstLdweights identical to the previous one (same AP, same mode) with
    no semaphore waits/updates is a no-op — remove it."""
    removed = 0
    for blk in nc.m.functions[0].blocks:
        prev_key = None
        warm_seen = False
        keep = []
        for inst in blk.instructions:
            nm = type(inst).__name__
            if nm == "InstLdweights":
                pap = inst.ins[0]
                key = (
                    pap.memref,
                    pap.offset,
                    str(pap.ap),
                    str(pap.dtype),
                    str(inst.perf_mode),
                    str(inst.is_transpose),
                )
                is_warm = "bfloat16" in str(pap.dtype)
                if (
                    key == prev_key
                    and not inst.has_wait()
                    and not inst.has_update()
                ):
                    # Identical to the previous load — or a warm-matmul
                    # weight reload: the warm matmul's output is scratch and
                    # its moving data is zeros, so it can run on whatever
                    # weights are resident; skipping the reload avoids
                    # clobbering the background weight buffer and exposing a
                    # serial LDWEIGHTS on the next real matmul.
                    removed += 1
                    continue
                if is_warm:
                    warm_seen = True
                prev_key = key
            keep.append(inst)
        if removed:
            blk.instructions = keep
    return removed


_PROGRAM_CACHE = None
_LDW_PATCHED = False


def _patch_ldw_opt():
    """Re-enable walrus's LDWEIGHTS elision (consecutive matmuls sharing a
    stationary tile skip the reload). bass_utils hardcodes it off."""
    global _LDW_PATCHED
    if _LDW_PATCHED:
        return
    import concourse.bass_utils as _bu

    _orig = _bu.run_command

    def _run(argv, **kwargs):
        if isinstance(argv, list):
            argv = [
                "--enable-ldw-opt=true" if a == "--enable-ldw-opt=false" else a
                for a in argv
            ]
        return _orig(argv, **kwargs)

    _bu.run_command = _run
    _LDW_PATCHED = True


def _get_program():
    global _PROGRAM_CACHE
    if _PROGRAM_CACHE is None:
        _PROGRAM_CACHE = _build_program()
    return _PROGRAM_CACHE


def _sign_bytes(a):
    """[R, K] fp32 -> [K, R] int8 view of each element's top byte (sign +
    high exponent bits). Negative byte <=> negative float. Pure memory
    marshalling — byte selection + transpose, no arithmetic."""
    b = a.view(np.uint8).reshape(a.shape[0], a.shape[1], 4)[:, :, 3]
    return np.ascontiguousarray(b.view(np.int8).T)


def _shard_inputs(x, y, x_clip, y_clip):
    x = np.ascontiguousarray(np.asarray(x, dtype=np.float32))
    y = np.ascontiguousarray(np.asarray(y, dtype=np.float32))
    clips = np.empty((P, 2), dtype=np.float32)
    clips[:, 0] = np.float32(x_clip)
    clips[:, 1] = np.float32(y_clip)
    xb = {}
    yb = {}
    for bb in range(B):
        for h in range(2):
            xb[(bb, h)] = _sign_bytes(x[bb, h * MSH : (h + 1) * MSH, :])
            yb[(bb, h)] = _sign_bytes(y[bb, h * NSH : (h + 1) * NSH, :])
    in_maps = []
    for c in range(NCORES):
        b, mh, nh = c // 4, (c % 4) // 2, c % 2
        in_maps.append(
            {"xsT": xb[(b, mh)], "ysT": yb[(b, nh)], "clips": clips}
        )
    return in_maps


def run_sharded(x, y, x_clip, y_clip, trace=False, **kwargs):
    """Run the SPMD kernel; returns (out, BassKernelResults)."""
    from concourse.bass_utils import run_bass_kernel_spmd

    nc = _get_program()
    in_maps = _shard_inputs(x, y, x_clip, y_clip)
    res = run_bass_kernel_spmd(
        nc, in_maps, core_ids=list(range(NCORES)), trace=trace, **kwargs
    )
    out = np.empty((B, M, N), dtype=np.float32)
    for c in range(NCORES):
        b, mh, nh = c // 4, (c % 4) // 2, c % 2
        out[b, mh * MSH : (mh + 1) * MSH, nh * NSH : (nh + 1) * NSH] = res.results[
            c
        ]["out"]
    return out, res


def kernel(x, y, x_clip, y_clip):
    out, _ = run_sharded(x, y, x_clip, y_clip, trace=False)
    return out


# revision 17
# speedup vs baseline: 1.0935x; 1.0211x over previous
# Binary (sign) matmul: out[b,m,n] = sum_k sign(x[b,m,k]) * sign(y[b,n,k]) * x_clip * y_clip
# B=2, M=N=K=4096, fp32 in/out.
#
# Sharding: 8 cores = batch(2) x 2x2 grid over (M, N). Each core computes a
# [2048, 2048] output block from x[b, mh*2048:, :] and y[b, nh*2048:, :].
#
# Host marshalling: only the sign-carrying high byte of each fp32 input
# element is shipped to the device (a pure byte-slice view + transpose —
# no arithmetic); the device computes sign() from it (the int8 byte is
# negative iff the fp32 value is negative) via the Sign activation, then
# runs fp8 DoubleRow matmuls. This cuts input DMA 4x (16MB/core vs 64MB)
# so the PE — not the DMA stream — is the pacing engine.
#
# Per-core device pipeline:
#   DMA int8 k-major sign bytes -> ScalarE Sign (int8 -> fp8e4 +-1, into
#   the matmul operand buffers) -> TensorE DoubleRow fp8 matmuls (exact:
#   sums of +-1 accumulate in fp32 PSUM) -> quarter-K spills: partial sums
#   accumulate exactly in an fp16 SBUF tensor (every partial is an even
#   integer <= 4096, exactly representable) -> DVE add + ScalarE clip
#   scale -> DMA out.
#
# Matmul schedule: K is split into 4 quarters of 4 DoubleRow steps. Each
# (m-tile, quarter) is one PSUM accumulation group of 4 banks; quarters
# are processed outer-to-inner in k-arrival order so the in-order PE queue
# never waits on a chunk that arrives later than runnable work. With only
# ~1/4 of K needed before the first matmuls close, the PE stays busy from
# ~30us onward instead of idling ~100us against the input stream.
import numpy as np

B = 2
M = N = K = 4096
P = 128
MSH, NSH = 2048, 2048      # per-core shard of M, N
KO = K // P                # 32 k-tiles of 128
MT = MSH // P              # 16 m row-tiles
FD = 512                   # matmul free dim
NCH = NSH // FD            # 4 n chunks
NCORES = 8
# K split into PSUM accumulation phases per m-tile: 4-step quarters
# (spilled exactly to fp16 ACC in k-arrival order so the PE never waits on
# the input stream), then one 8-step half (drained straight to the output,
# half as many drains so the out-DMA chain hides under the matmuls).
#
# The first FINE m-tiles additionally split their first quarter into two
# 2-step sub-phases: the PE's 64-deep reorder window can then reach enough
# runnable low-kd matmuls to cover the Sign stream's ramp (kd j is fully
# signed only at ~18+7.6j us), eliminating ~10us of early pinch stalls.
FINE = 6
SEGMENTS = (
    (0, FINE, 0, 2, "cast"),    # A1: fine tiles, kd 0-1
    (FINE, MT, 0, 4, "cast"),   # A:  remaining tiles, kd 0-3
    (0, FINE, 2, 2, "add"),     # A2: fine tiles, kd 2-3
    (0, MT, 4, 4, "add"),       # B:  all tiles, kd 4-7
    (0, MT, 8, 8, "final"),     # C:  all tiles, kd 8-15, drain to output
)


def _build_program():
    import concourse.bacc as bacc
    import concourse.mybir as mybir
    import concourse.tile as tile
    from concourse.bass import ts

    f32 = mybir.dt.float32
    f16 = mybir.dt.float16
    bf16 = mybir.dt.bfloat16
    i8 = mybir.dt.int8
    op_dt = mybir.dt.float8e4
    Sign = mybir.ActivationFunctionType.Sign
    Copy = mybir.ActivationFunctionType.Copy

    # Bacc (not bass.Bass): its compile() legalizes multi-sem waits into
    # event-semaphore carriers — TRN2 instructions support only 1 HW wait.
    nc = bacc.Bacc(
        "TRN2",
        target_bir_lowering=False,
        debug=False,
        num_devices=NCORES,
    )
    xsT = nc.dram_tensor("xsT", [K, MSH], i8, kind="ExternalInput").ap()
    ysT = nc.dram_tensor("ysT", [K, NSH], i8, kind="ExternalInput").ap()
    clips = nc.dram_tensor("clips", [P, 2], f32, kind="ExternalInput").ap()
    out = nc.dram_tensor("out", [MSH, NSH], f32, kind="ExternalOutput").ap()

    with tile.TileContext(nc) as tc:
        with (
            tc.tile_pool(name="constp", bufs=1) as constp,
            tc.tile_pool(name="sytp", bufs=1) as sytp,
            tc.tile_pool(name="sxtp", bufs=1) as sxtp,
            tc.tile_pool(name="accp", bufs=1) as accp,
            tc.tile_pool(name="stagep", bufs=3) as stagep,
            tc.tile_pool(name="outp", bufs=4) as outp,
            tc.tile_pool(name="psump", bufs=7, space="PSUM") as psump,
            tc.tile_pool(name="dpsump", bufs=1, space="PSUM") as dpsump,
        ):
            # clip product, replicated per-partition: [P, 1]
            clip_sb = constp.tile([P, 2], f32)
            nc.sync.dma_start(clip_sb[:], clips)
            clip_prod = constp.tile([P, 1], f32)
            nc.vector.tensor_tensor(
                clip_prod[:], clip_sb[:, 0:1], clip_sb[:, 1:2],
                mybir.AluOpType.mult,
            )

            # SxT[ki, ko, m] = sign(x[m, ko*P + ki]); SyT likewise for y.
            SxT = sxtp.tile([P, KO, MSH], op_dt)
            SyT = sytp.tile([P, KO, NSH], op_dt)

            # HAM warmth: DoubleRow matmuls don't register as PE activity,
            # so the clock gate re-throttles to 1.2 GHz mid-stream. A tiny
            # normal-mode matmul on scratch data every couple of k-steps
            # keeps the activity monitor busy; it clobbers 32 stationary
            # columns, so it only ever sits at k-step boundaries (weights
            # reload anyway).
            dwarm = constp.tile([P, 32], bf16)
            nc.vector.memset(dwarm[:], 0)

            def warm_mm():
                dps = dpsump.tile([32, 32], f32, name="dps", tag="dps")
                nc.tensor.matmul(
                    dps[:], lhsT=dwarm[:, :32], rhs=dwarm[:], start=True,
                    stop=True,
                )

            def prep(src_dram, ko, dst):
                st = stagep.tile([P, MSH], i8, name="st", tag="stage")
                nc.sync.dma_start(st[:], src_dram[ts(ko, P), :])
                nc.scalar.activation(dst, st[:], Sign)

            # Stream k-chunks: x and y interleaved so the first matmuls can
            # start after one chunk of each; the Tile scheduler overlaps the
            # rest of the prep with the matmul wavefront.
            for ko in range(KO):
                prep(xsT, ko, SxT[:, ko, :])
                prep(ysT, ko, SyT[:, ko, :])

            # Ramp the PE p-state before real work hits it.
            for _ in range(6):
                warm_mm()

            # Exact fp16 accumulator for quarter partial sums: every spilled
            # value is an even integer of magnitude <= 4096.
            ACC = accp.tile([P, MT, NCH, FD], f16, name="ACC")

            kstep = 0
            for i0, i1, kd0, nkd, kind in SEGMENTS:
                for i in range(i0, i1):
                    pss = [
                        psump.tile([P, FD], f32, name=f"ps{n}", tag="ps")
                        for n in range(NCH)
                    ]
                    for kdq in range(nkd):
                        kd = kd0 + kdq
                        # One warm matmul every 4th k-step (~4.2us) keeps the
                        # HAM clock gate near its full-speed window at
                        # minimum PE cost.
                        if kstep % 4 == 2:
                            warm_mm()
                        kstep += 1
                        # nch-inner: 4 consecutive matmuls share one
                        # stationary tile; _dedupe_ldweights drops the
                        # redundant loads.
                        for nch in range(NCH):
                            nc.tensor.matmul(
                                pss[nch][:],
                                lhsT=SxT[:, 2 * kd : 2 * kd + 2, ts(i, P)],
                                rhs=SyT[:, 2 * kd : 2 * kd + 2, ts(nch, FD)],
                                start=(kdq == 0),
                                stop=(kdq == nkd - 1),
                                perf_mode=mybir.MatmulPerfMode.DoubleRow,
                            )
                    for nch in range(NCH):
                        if kind == "cast":
                            nc.vector.tensor_copy(
                                out=ACC[:, i, nch, :], in_=pss[nch][:]
                            )
                        elif kind == "add":
                            nc.vector.tensor_tensor(
                                ACC[:, i, nch, :], pss[nch][:],
                                ACC[:, i, nch, :], mybir.AluOpType.add,
                            )
                        else:
                            ot = outp.tile([P, FD], f32, name="ot")
                            nc.vector.tensor_tensor(
                                ot[:], pss[nch][:], ACC[:, i, nch, :],
                                mybir.AluOpType.add,
                            )
                            # clip scale on the Scalar engine (its queue is
                            # idle once the signs are done); DVE only does
                            # the adds.
                            nc.scalar.activation(
                                ot[:], ot[:], Copy, scale=clip_prod[:]
                            )
                            nc.sync.dma_start(
                                out[ts(i, P), ts(nch, FD)], ot[:]
                            )

    nc.compile()
    _dedupe_ldweights(nc)
    return nc


def _dedupe_ldweights(nc):
    """Drop redundant standalone InstLdweights left by bacc's matmul split.

    With the nch-inner loop order, 4 consecutive matmuls share one
    stationary tile; bacc still emits one InstLdweights per matmul. An
    InstLdweights identical to the previous one (same AP, same mode) with
    no semaphore waits/updates is a no-op — remove it."""
    removed = 0
    for blk in nc.m.functions[0].blocks:
        prev_key = None
        warm_seen = False
        keep = []
        for inst in blk.instructions:
            nm = type(inst).__name__
            if nm == "InstLdweights":
                pap = inst.ins[0]
                key = (
                    pap.memref,
                    pap.offset,
                    str(pap.ap),
                    str(pap.dtype),
                    str(inst.perf_mode),
                    str(inst.is_transpose),
                )
                is_warm = "bfloat16" in str(pap.dtype)
                if (
                    key == prev_key
                    and not inst.has_wait()
                    and not inst.has_update()
                ):
                    # Identical to the previous load — or a warm-matmul
                    # weight reload: the warm matmul's output is scratch and
                    # its moving data is zeros, so it can run on whatever
                    # weights are resident; skipping the reload avoids
                    # clobbering the background weight buffer and exposing a
                    # serial LDWEIGHTS on the next real matmul.
                    removed += 1
                    continue
                if is_warm:
                    warm_seen = True
                prev_key = key
            keep.append(inst)
        if removed:
            blk.instructions = keep
    return removed


_PROGRAM_CACHE = None
_LDW_PATCHED = False


def _patch_ldw_opt():
    """Re-enable walrus's LDWEIGHTS elision (consecutive matmuls sharing a
    stationary tile skip the reload). bass_utils hardcodes it off."""
    global _LDW_PATCHED
    if _LDW_PATCHED:
        return
    import concourse.bass_utils as _bu

    _orig = _bu.run_command

    def _run(argv, **kwargs):
        if isinstance(argv, list):
            argv = [
                "--enable-ldw-opt=true" if a == "--enable-ldw-opt=false" else a
                for a in argv
            ]
        return _orig(argv, **kwargs)

    _bu.run_command = _run
    _LDW_PATCHED = True


def _get_program():
    global _PROGRAM_CACHE
    if _PROGRAM_CACHE is None:
        _PROGRAM_CACHE = _build_program()
    return _PROGRAM_CACHE


def _sign_bytes(a):
    """[R, K] fp32 -> [K, R] int8 view of each element's top byte (sign +
    high exponent bits). Negative byte <=> negative float. Pure memory
    marshalling — byte selection + transpose, no arithmetic."""
    b = a.view(np.uint8).reshape(a.shape[0], a.shape[1], 4)[:, :, 3]
    return np.ascontiguousarray(b.view(np.int8).T)


def _shard_inputs(x, y, x_clip, y_clip):
    x = np.ascontiguousarray(np.asarray(x, dtype=np.float32))
    y = np.ascontiguousarray(np.asarray(y, dtype=np.float32))
    clips = np.empty((P, 2), dtype=np.float32)
    clips[:, 0] = np.float32(x_clip)
    clips[:, 1] = np.float32(y_clip)
    xb = {}
    yb = {}
    for bb in range(B):
        for h in range(2):
            xb[(bb, h)] = _sign_bytes(x[bb, h * MSH : (h + 1) * MSH, :])
            yb[(bb, h)] = _sign_bytes(y[bb, h * NSH : (h + 1) * NSH, :])
    in_maps = []
    for c in range(NCORES):
        b, mh, nh = c // 4, (c % 4) // 2, c % 2
        in_maps.append(
            {"xsT": xb[(b, mh)], "ysT": yb[(b, nh)], "clips": clips}
        )
    return in_maps


def run_sharded(x, y, x_clip, y_clip, trace=False, **kwargs):
    """Run the SPMD kernel; returns (out, BassKernelResults)."""
    from concourse.bass_utils import run_bass_kernel_spmd

    nc = _get_program()
    in_maps = _shard_inputs(x, y, x_clip, y_clip)
    res = run_bass_kernel_spmd(
        nc, in_maps, core_ids=list(range(NCORES)), trace=trace, **kwargs
    )
    out = np.empty((B, M, N), dtype=np.float32)
    for c in range(NCORES):
        b, mh, nh = c // 4, (c % 4) // 2, c % 2
        out[b, mh * MSH : (mh + 1) * MSH, nh * NSH : (nh + 1) * NSH] = res.results[
            c
        ]["out"]
    return out, res


def kernel(x, y, x_clip, y_clip):
    out, _ = run_sharded(x, y, x_clip, y_clip, trace=False)
    return out
